# revision 34
# baseline (speedup 1.0000x reference)
"""Trainium2 Bass kernel for nn_CondensedEmbracementLayer.

Computation (per batch element b):
  prefix[b]  = number of leading 1s in attention_mask[b]  (contiguous-prefix mask)
  n_valid[b] = max(prefix[b] - 1, 1)
  u          = jax.random.uniform(key(42), (32, 1024))      # input-independent constant
  idx[b, j]  = min(trunc_f32(u[b, j] * n_valid[b]), n_valid[b] - 1)
  out[b, j]  = tokens[b, idx[b, j], j]

Strategy: pure data parallel over batch — 4 batch elements per NeuronCore x 8
cores. On each core the mask is popcounted on-chip (free-dim reduce +
partition all-reduce), the sample row indices are computed in f32 exactly
(all intermediates are integers < 2^24; floor via the +2^23 round trick plus
an is_gt fixup so it bit-matches jax's truncation), and the 4096 needed
elements are fetched directly from HBM with indirect/gather DMAs — the 64MB
token shard is never streamed.

Two device-side gather flavours:
  v2: 32 x indirect_dma_start, one [128,1] element-gather per column
      (HW emits one descriptor per partition).
  v3: 16 x dma_gather, 256 row-chunks of 64 f32 each per instruction
      (int16 row indices wrapped in 16 partitions), then a constant-mask
      multiply + free-dim reduce extracts the wanted element per chunk.
"""

import os
import numpy as np

BS, SEQ, HID = 32, 4096, 1024
NCORES = 8
BPC = BS // NCORES          # batch elements per core
JW = HID // 128             # j-values per partition per batch element (v2)
FREE = BPC * JW             # free-dim columns of the v2 work tile
TWO23 = float(2 << 22)

# v3 geometry: 8 column blocks of 128; per block 512 (b, j) pairs.
# NGRP == BPC lets the pair mapping be (b = grp, c = pd), which keeps every
# access pattern at <=3 dims (walrus codegen can't encode 4-D+ patterns).
NBLK = 8
CW = HID // NBLK            # 128 columns per block
NPAIR = BPC * CW            # 512 gathered rows per block
NGRP = NPAIR // 128         # 4 dest partition groups per block (== BPC)
SBLK = NPAIR // 16          # 32 index-tile columns per block

KVER = int(os.environ.get("BASS_KERNEL_V", "3"))

_CACHE = {}


def _prefix_pipeline(nc, pool, psum_pool, mask, ones128, mybir, bass_isa):
    """mask [BPC, SEQ] -> nv, nm1 tiles [128, BPC] (broadcast to all parts).

    The cross-partition popcount reduction + broadcast is one ones-matmul on
    the otherwise idle TensorE: out[p, b] = sum_k ones[k, p] * part[k, b]
    (gpsimd partition_all_reduce measured ~10x slower on HW).
    """
    f32 = mybir.dt.float32
    i32 = mybir.dt.int32
    # partition p holds seq chunk [p*32, (p+1)*32) of each batch row, so the
    # load uses contiguous 128B runs instead of 4B-strided descriptors
    m_i = pool.tile([128, BPC * 32], i32)
    nc.sync.dma_start(
        out=m_i[:].rearrange("p (b c) -> p b c", b=BPC),
        in_=mask.ap().rearrange("b (p c) -> p b c", p=128),
    )
    part = pool.tile([128, BPC], f32)
    nc.vector.tensor_reduce(
        out=part[:],
        in_=m_i[:].rearrange("p (b c) -> p b c", b=BPC),
        axis=mybir.AxisListType.X,
        op=mybir.AluOpType.add,
    )
    if ones128 is not None:
        pref_ps = psum_pool.tile([128, BPC], f32, space="PSUM")
        nc.tensor.matmul(
            out=pref_ps[:], lhsT=ones128, rhs=part[:], start=True, stop=True
        )
        pref_ap = pref_ps[:]
    else:
        pref = pool.tile([128, BPC], f32)
        nc.gpsimd.partition_all_reduce(
            pref[:], part[:], channels=128, reduce_op=bass_isa.ReduceOp.add
        )
        pref_ap = pref[:]
    nv = pool.tile([128, BPC], f32)
    nc.vector.tensor_scalar(
        out=nv[:], in0=pref_ap, scalar1=-1.0, scalar2=1.0,
        op0=mybir.AluOpType.add, op1=mybir.AluOpType.max,
    )
    # nm1 = n_valid - 1 = max(prefix - 2, 0), computed from pref directly so
    # it doesn't serialize behind nv
    nm1 = pool.tile([128, BPC], f32)
    nc.vector.tensor_scalar(
        out=nm1[:], in0=pref_ap, scalar1=-2.0, scalar2=0.0,
        op0=mybir.AluOpType.add, op1=mybir.AluOpType.max,
    )
    return nv, nm1


def _build_module_v2():
    from concourse import bacc, bass, bass_isa, mybir
    from concourse.tile import TileContext

    f32 = mybir.dt.float32
    i32 = mybir.dt.int32
    nc = bacc.Bacc("TRN2", target_bir_lowering=False, debug=False,
                   enable_asserts=False, num_devices=NCORES)
    tokens = nc.dram_tensor("tokens", [BPC, SEQ, HID], f32, kind="ExternalInput")
    mask = nc.dram_tensor("mask", [BPC, SEQ], i32, kind="ExternalInput")
    u_l = nc.dram_tensor("u_l", [128, FREE], f32, kind="ExternalInput")
    addc = nc.dram_tensor("addc", [128, FREE], f32, kind="ExternalInput")
    out = nc.dram_tensor("out", [BPC, HID], f32, kind="ExternalOutput")

    with TileContext(nc) as tc:
        with tc.tile_pool(name="p", bufs=1) as pool:
            nv, nm1 = _prefix_pipeline(nc, pool, None, mask, None, mybir, bass_isa)
            u_t = pool.tile([128, FREE], f32)
            nc.sync.dma_start(out=u_t[:], in_=u_l.ap())
            ac_t = pool.tile([128, FREE], f32)
            nc.sync.dma_start(out=ac_t[:], in_=addc.ap())

            def bcast(t):
                return t[:].rearrange("p (b o) -> p b o", o=1).to_broadcast(
                    [128, BPC, JW]
                )

            y = pool.tile([128, FREE], f32)
            nc.vector.tensor_tensor(
                out=y[:].rearrange("p (b t) -> p b t", b=BPC),
                in0=u_t[:].rearrange("p (b t) -> p b t", b=BPC),
                in1=bcast(nv), op=mybir.AluOpType.mult,
            )
            r = pool.tile([128, FREE], f32)
            nc.vector.tensor_scalar(
                out=r[:], in0=y[:], scalar1=TWO23, scalar2=-TWO23,
                op0=mybir.AluOpType.add, op1=mybir.AluOpType.add,
            )
            g = pool.tile([128, FREE], f32)
            nc.vector.tensor_tensor(out=g[:], in0=r[:], in1=y[:],
                                    op=mybir.AluOpType.is_gt)
            fl = pool.tile([128, FREE], f32)
            nc.vector.tensor_tensor(out=fl[:], in0=r[:], in1=g[:],
                                    op=mybir.AluOpType.subtract)
            im = pool.tile([128, FREE], f32)
            nc.vector.tensor_tensor(
                out=im[:].rearrange("p (b t) -> p b t", b=BPC),
                in0=fl[:].rearrange("p (b t) -> p b t", b=BPC),
                in1=bcast(nm1), op=mybir.AluOpType.min,
            )
            ef = pool.tile([128, FREE], f32)
            nc.vector.tensor_scalar_mul(out=ef[:], in0=im[:], scalar1=float(HID))
            ei_f = pool.tile([128, FREE], f32)
            nc.vector.tensor_tensor(out=ei_f[:], in0=ef[:], in1=ac_t[:],
                                    op=mybir.AluOpType.add)
            eidx = pool.tile([128, FREE], i32)
            nc.vector.tensor_copy(out=eidx[:], in_=ei_f[:])

            gt = pool.tile([128, FREE], f32)
            tok_flat = tokens.ap().rearrange("a b (c o) -> (a b c) o", o=1)
            for k in range(FREE):
                nc.gpsimd.indirect_dma_start(
                    out=gt[:, k : k + 1],
                    out_offset=None,
                    in_=tok_flat,
                    in_offset=bass.IndirectOffsetOnAxis(
                        ap=eidx[:, k : k + 1], axis=0
                    ),
                )
            nc.sync.dma_start(
                out=out.ap().rearrange("b (p t) -> p b t", t=JW),
                in_=gt[:].rearrange("p (b t) -> p b t", b=BPC),
            )
    nc.compile()
    return nc


def _v3_body(nc, tc, pool, psum_pool, tokens, mask, consts, out, mybir, bass_isa):
    f32 = mybir.dt.float32
    i16 = mybir.dt.int16
    COLS = NBLK * SBLK  # 256
    if True:
        if True:
            # one fused constant load: [u_r | badd | emask | ones]
            cst = pool.tile([128, 2 * COLS + CW + 128], f32)
            nc.sync.dma_start(out=cst[:], in_=consts.ap())
            u_t = cst[:, 0:COLS]
            ba_t = cst[:, COLS : 2 * COLS]
            m_t = cst[:, 2 * COLS : 2 * COLS + CW]
            ones128 = cst[:, 2 * COLS + CW : 2 * COLS + CW + 128]
            nv, nm1 = _prefix_pipeline(
                nc, pool, psum_pool, mask, ones128, mybir, bass_isa
            )

            # materialize n / n-1 in the 32-column period (col%32 = s, b=s//8):
            # n32[p, s] = nv[p, s//8]; ops then broadcast it across the 8 blocks
            n32 = pool.tile([128, SBLK], f32)
            nc.vector.tensor_copy(
                out=n32[:].rearrange("p (b t) -> p b t", b=BPC),
                in_=nv[:].rearrange("p (b o) -> p b o", o=1).to_broadcast(
                    [128, BPC, SBLK // BPC]
                ),
            )
            # nb32 = (n-1) + b*SEQ in the 32-column period (badd period is 32)
            nb32 = pool.tile([128, SBLK], f32)
            nc.vector.tensor_tensor(
                out=nb32[:].rearrange("p (b t) -> p b t", b=BPC),
                in0=nm1[:].rearrange("p (b o) -> p b o", o=1).to_broadcast(
                    [128, BPC, SBLK // BPC]
                ),
                in1=ba_t[:, :SBLK].rearrange("p (b t) -> p b t", b=BPC),
                op=mybir.AluOpType.add,
            )

            def brep(t):  # [128, 32] -> [128, NBLK, 32] (repeat across blocks)
                return t[:].rearrange("p (o s) -> p o s", o=1).to_broadcast(
                    [128, NBLK, SBLK]
                )

            def v3d(t):  # [128, 256] -> [128, NBLK, 32]
                return t[:].rearrange("p (blk s) -> p blk s", blk=NBLK)

            y = pool.tile([128, COLS], f32)
            nc.vector.tensor_tensor(out=v3d(y), in0=v3d(u_t), in1=brep(n32),
                                    op=mybir.AluOpType.mult)
            r = pool.tile([128, COLS], f32)
            nc.vector.tensor_scalar(
                out=r[:], in0=y[:], scalar1=TWO23, scalar2=-TWO23,
                op0=mybir.AluOpType.add, op1=mybir.AluOpType.add,
            )
            g = pool.tile([128, COLS], f32)
            nc.vector.tensor_tensor(out=g[:], in0=r[:], in1=y[:],
                                    op=mybir.AluOpType.is_gt)
            fl = pool.tile([128, COLS], f32)
            nc.vector.tensor_tensor(out=fl[:], in0=r[:], in1=g[:],
                                    op=mybir.AluOpType.subtract)
            # row = min(floor + b*SEQ, (n-1) + b*SEQ), emitted directly as
            # int16 (exact-integer convert) — the gather's index dtype
            fl2 = pool.tile([128, COLS], f32)
            nc.vector.tensor_tensor(out=fl2[:], in0=fl[:], in1=ba_t[:],
                                    op=mybir.AluOpType.add)
            t16 = pool.tile([128, COLS], i16)
            nc.vector.tensor_tensor(out=v3d(t16), in0=v3d(fl2), in1=brep(nb32),
                                    op=mybir.AluOpType.min)

            # NBLK gathers, each followed by its own extraction so the DVE
            # work overlaps the Pool-serialized descriptor generation.
            # Extraction: multiply by the constant one-hot mask (cc == pd) and
            # reduce over the chunk dim.
            tok2d = tokens.ap().rearrange("a s h -> (a s) h")
            e_t = pool.tile([128, NBLK * NGRP], f32)
            mview = m_t[:].rearrange("p (o c) -> p o c", o=1).to_broadcast(
                [128, NGRP, CW]
            )
            for blk in range(NBLK):
                gk = pool.tile([128, NGRP * CW], f32, tag=f"g{blk}")
                nc.gpsimd.dma_gather(
                    out_ap=gk[:].rearrange("p (g c) -> p g c", c=CW),
                    in_ap=tok2d[:, blk * CW : (blk + 1) * CW],
                    idxs_ap=t16[:, blk * SBLK : (blk + 1) * SBLK],
                    num_idxs=NPAIR,
                    num_idxs_reg=NPAIR,
                    elem_size=CW,
                    elem_step=HID,
                )
                pk = pool.tile([128, NGRP * CW], f32, tag=f"pk{blk}")
                nc.vector.tensor_tensor(
                    out=pk[:].rearrange("p (g c) -> p g c", c=CW),
                    in0=gk[:].rearrange("p (g c) -> p g c", c=CW),
                    in1=mview,
                    op=mybir.AluOpType.mult,
                )
                nc.vector.tensor_reduce(
                    out=e_t[:, blk * NGRP : (blk + 1) * NGRP],
                    in_=pk[:].rearrange("p (g c) -> p g c", c=CW),
                    axis=mybir.AxisListType.X,
                    op=mybir.AluOpType.add,
                )
            # store: E[pd, blk*NGRP+b] -> out[b, blk*CW + pd]
            for b in range(BPC):
                nc.sync.dma_start(
                    out=out.ap()[b].rearrange("(blk pd) -> pd blk", pd=CW),
                    in_=e_t[:, b :: NGRP],
                )


def _build_module_v3(reps=1):
    from concourse import bacc, bass_isa, mybir
    from concourse.tile import TileContext

    f32 = mybir.dt.float32
    i32 = mybir.dt.int32
    nc = bacc.Bacc("TRN2", target_bir_lowering=False, debug=False,
                   enable_asserts=False, num_devices=NCORES)
    tokens = nc.dram_tensor("tokens", [BPC, SEQ, HID], f32, kind="ExternalInput")
    mask = nc.dram_tensor("mask", [BPC, SEQ], i32, kind="ExternalInput")
    # pair mapping within a block: pair i -> dest (pd = i%128, grp = i//128),
    # batch b = grp, column c = pd, j = blk*CW + pd.
    # index-tile slot (p0 = p%16, s = i//16):  i = s*16 + p0, so b = s//8 and
    # c = (s%8)*16 + p0 — b varies along a clean column axis (col = blk*32+s).
    # fused constants [u_r | badd | emask]:
    #   u_r[p, blk*SBLK+s]  = u[4*core + b(i), j(i, blk)]
    #   badd[p, blk*SBLK+s] = b(i) * SEQ
    #   emask[pd, cc]       = (cc == pd)
    consts = nc.dram_tensor(
        "consts", [128, 2 * NBLK * SBLK + CW + 128], f32, kind="ExternalInput"
    )
    out = nc.dram_tensor("out", [BPC, HID], f32, kind="ExternalOutput")

    with TileContext(nc) as tc:
        for _ in range(reps):
            with tc.tile_pool(name="p", bufs=1) as pool, tc.tile_pool(
                name="ps", bufs=1, space="PSUM"
            ) as psum_pool:
                _v3_body(
                    nc, tc, pool, psum_pool, tokens, mask, consts, out,
                    mybir, bass_isa,
                )
    nc.compile()
    return nc


def _get_module():
    if "nc" not in _CACHE:
        _CACHE["nc"] = _build_module_v3() if KVER == 3 else _build_module_v2()
    return _CACHE["nc"]


def _u_const():
    # Input-independent sampling constant; bit-identical to the reference's
    # jax.random.uniform(key(42), ...) on any backend (threefry is
    # platform-deterministic).
    if "u" not in _CACHE:
        import jax

        with jax.default_device(jax.devices("cpu")[0]):
            u = jax.random.uniform(
                jax.random.key(42), (BS, HID), dtype="float32"
            )
            _CACHE["u"] = np.asarray(u)
    return _CACHE["u"]


def _consts_v2():
    if "addc" not in _CACHE:
        b = np.arange(BPC, dtype=np.int64)[None, :, None] * (SEQ * HID)
        j = (
            np.arange(128, dtype=np.int64)[:, None, None] * JW
            + np.arange(JW, dtype=np.int64)[None, None, :]
        )
        _CACHE["addc"] = (b + j).reshape(128, FREE).astype(np.float32)
    return _CACHE["addc"]


def _pair_decode(p, col):
    """Index-tile slot (partition p, column col=blk*SBLK+s) -> (b, j)."""
    blk, s = col // SBLK, col % SBLK
    i = s * 16 + (p % 16)
    pd, grp = i % 128, i // 128
    return grp, blk * CW + pd


def _consts_v3():
    if "badd" not in _CACHE:
        p = np.arange(128)[:, None]
        col = np.arange(NBLK * SBLK)[None, :]
        b, _ = _pair_decode(p, col)
        _CACHE["badd"] = (b * SEQ).astype(np.float32)
        pd = np.arange(128)[:, None]
        cc = np.arange(CW)[None, :]
        _CACHE["emask"] = (cc == pd).astype(np.float32)
    return _CACHE["badd"], _CACHE["emask"]


def _u_r_for_core(u, core):
    p = np.arange(128)[:, None]
    col = np.arange(NBLK * SBLK)[None, :]
    b, j = _pair_decode(p, col)
    return np.ascontiguousarray(u[4 * core + b, j].astype(np.float32))


def build_in_maps(tokens, mask):
    u = _u_const()
    in_maps = []
    for c in range(NCORES):
        sl = slice(c * BPC, (c + 1) * BPC)
        m = {
            "tokens": np.ascontiguousarray(tokens[sl], dtype=np.float32),
            "mask": np.ascontiguousarray(mask[sl], dtype=np.int32),
        }
        if KVER == 3:
            badd, emask = _consts_v3()
            ones = np.ones((128, 128), np.float32)
            m["consts"] = np.ascontiguousarray(
                np.concatenate([_u_r_for_core(u, c), badd, emask, ones], axis=1)
            )
        else:
            u_c = (
                u[sl].reshape(BPC, 128, JW).transpose(1, 0, 2).reshape(128, FREE)
            )
            m["u_l"] = np.ascontiguousarray(u_c)
            m["addc"] = _consts_v2()
        in_maps.append(m)
    return in_maps


def _get_runner():
    """Build the 8-core shard_map jit once; returns f(in_maps) -> [out_c]."""
    if "runner" in _CACHE:
        return _CACHE["runner"]
    import jax
    from jax.sharding import Mesh, NamedSharding, PartitionSpec
    from jax.experimental.shard_map import shard_map
    from concourse import mybir
    from concourse.bass2jax import (
        _bass_exec_p,
        install_neuronx_cc_hook,
        partition_id_tensor,
    )

    nc = _get_module()
    install_neuronx_cc_hook()
    partition_name = nc.partition_id_tensor.name if nc.partition_id_tensor else None
    in_names, out_names, out_avals, zero_outs = [], [], [], []
    for alloc in nc.m.functions[0].allocations:
        if type(alloc).__name__ != "MemoryLocationSet":
            continue
        name = alloc.memorylocations[0].name
        if alloc.kind == "ExternalInput":
            if name != partition_name:
                in_names.append(name)
        elif alloc.kind == "ExternalOutput":
            out_names.append(name)
            shape = tuple(alloc.tensor_shape)
            dtype = mybir.dt.np(alloc.dtype)
            out_avals.append(jax.core.ShapedArray(shape, dtype))
            zero_outs.append(np.zeros(shape, dtype))
    n_params = len(in_names)
    all_in_names = in_names + out_names
    if partition_name is not None:
        all_in_names = all_in_names + [partition_name]

    def _body(*args):
        operands = list(args)
        if partition_name is not None:
            operands.append(partition_id_tensor())
        return tuple(
            _bass_exec_p.bind(
                *operands,
                out_avals=tuple(out_avals),
                in_names=tuple(all_in_names),
                out_names=tuple(out_names),
                lowering_input_output_aliases=(),
                sim_require_finite=True,
                sim_require_nnan=True,
                nc=nc,
            )
        )

    devices = jax.devices()[:NCORES]
    mesh = Mesh(np.asarray(devices), ("core",))
    n_outs = len(out_names)
    sharded = jax.jit(
        shard_map(
            _body,
            mesh=mesh,
            in_specs=(PartitionSpec("core"),) * (n_params + n_outs),
            out_specs=(PartitionSpec("core"),) * n_outs,
            check_rep=False,
        ),
        donate_argnums=tuple(range(n_params, n_params + n_outs)),
        keep_unused=True,
    )
    sh = NamedSharding(mesh, PartitionSpec("core"))

    def run(in_maps):
        concat_in = [
            np.concatenate(
                [np.asarray(in_maps[c][nm]) for c in range(NCORES)], axis=0
            )
            for nm in in_names
        ]
        zeros = [
            np.zeros((NCORES * z.shape[0], *z.shape[1:]), z.dtype)
            for z in zero_outs
        ]
        out_arrs = sharded(*concat_in, *zeros)
        oi = out_names.index("out")
        full = np.asarray(out_arrs[oi])
        return full.reshape(NCORES, *out_avals[oi].shape)

    _CACHE["runner_parts"] = {
        "sharded": sharded,
        "in_names": in_names,
        "out_names": out_names,
        "zero_outs": zero_outs,
        "mesh": mesh,
    }
    _CACHE["runner"] = run
    return run


def kernel(output_tokens_from_bert, attention_mask):
    run = _get_runner()
    in_maps = build_in_maps(output_tokens_from_bert, attention_mask)
    per_core = run(in_maps)
    return np.concatenate(list(per_core), axis=0).astype(np.float32)


# revision 36
# speedup vs baseline: 1.1247x; 1.1247x over previous
"""Trainium2 Bass kernel for nn_CondensedEmbracementLayer.

Computation (per batch element b):
  prefix[b]  = number of leading 1s in attention_mask[b]  (contiguous-prefix mask)
  n_valid[b] = max(prefix[b] - 1, 1)
  u          = jax.random.uniform(key(42), (32, 1024))      # input-independent constant
  idx[b, j]  = min(trunc_f32(u[b, j] * n_valid[b]), n_valid[b] - 1)
  out[b, j]  = tokens[b, idx[b, j], j]

Strategy: pure data parallel over batch — 4 batch elements per NeuronCore x 8
cores. On each core the mask is popcounted on-chip (free-dim reduce +
partition all-reduce), the sample row indices are computed in f32 exactly
(all intermediates are integers < 2^24; floor via the +2^23 round trick plus
an is_gt fixup so it bit-matches jax's truncation), and the 4096 needed
elements are fetched directly from HBM with indirect/gather DMAs — the 64MB
token shard is never streamed.

Two device-side gather flavours (BASS_KERNEL_V, default 3):
  v2: 32 x indirect_dma_start, one [128,1] element-gather per column
      (the HW ucode emits exactly one descriptor per partition).
  v3: 8 x dma_gather, 512 row-chunks of 128 f32 each per instruction
      (int16 row indices wrapped in 16 partitions), each followed by a
      constant-one-hot multiply + free-dim reduce that extracts the wanted
      element per chunk while the next gather's descriptors generate.
      The SWDGE scratch ring is enlarged to 128KB so consecutive gathers'
      descriptors pipeline instead of serializing on ring space, and the
      cross-partition popcount reduce+broadcast is a single ones-matmul on
      the otherwise idle TensorE.
"""

import os
import numpy as np

BS, SEQ, HID = 32, 4096, 1024
NCORES = 8
BPC = BS // NCORES          # batch elements per core
JW = HID // 128             # j-values per partition per batch element (v2)
FREE = BPC * JW             # free-dim columns of the v2 work tile
TWO23 = float(2 << 22)

# v3 geometry: 8 column blocks of 128; per block 512 (b, j) pairs.
# NGRP == BPC lets the pair mapping be (b = grp, c = pd), which keeps every
# access pattern at <=3 dims (walrus codegen can't encode 4-D+ patterns).
NBLK = 8
CW = HID // NBLK            # 128 columns per block
NPAIR = BPC * CW            # 512 gathered rows per block
NGRP = NPAIR // 128         # 4 dest partition groups per block (== BPC)
SBLK = NPAIR // 16          # 32 index-tile columns per block

KVER = int(os.environ.get("BASS_KERNEL_V", "3"))

_CACHE = {}


def _prefix_pipeline(nc, pool, psum_pool, mask, ones128, mybir, bass_isa):
    """mask [BPC, SEQ] -> nv, nm1 tiles [128, BPC] (broadcast to all parts).

    The cross-partition popcount reduction + broadcast is one ones-matmul on
    the otherwise idle TensorE: out[p, b] = sum_k ones[k, p] * part[k, b]
    (gpsimd partition_all_reduce measured ~10x slower on HW).
    """
    f32 = mybir.dt.float32
    i32 = mybir.dt.int32
    # partition p holds seq chunk [p*32, (p+1)*32) of each batch row, so the
    # load uses contiguous 128B runs instead of 4B-strided descriptors
    m_i = pool.tile([128, BPC * 32], i32)
    nc.sync.dma_start(
        out=m_i[:].rearrange("p (b c) -> p b c", b=BPC),
        in_=mask.ap().rearrange("b (p c) -> p b c", p=128),
    )
    m_f = pool.tile([128, BPC * 32], f32)
    nc.vector.tensor_copy(out=m_f[:], in_=m_i[:])
    part = pool.tile([128, BPC], f32)
    nc.vector.tensor_reduce(
        out=part[:],
        in_=m_f[:].rearrange("p (b c) -> p b c", b=BPC),
        axis=mybir.AxisListType.X,
        op=mybir.AluOpType.add,
    )
    if ones128 is not None:
        pref_ps = psum_pool.tile([128, BPC], f32, space="PSUM")
        nc.tensor.matmul(
            out=pref_ps[:], lhsT=ones128, rhs=part[:], start=True, stop=True
        )
        pref_ap = pref_ps[:]
    else:
        pref = pool.tile([128, BPC], f32)
        nc.gpsimd.partition_all_reduce(
            pref[:], part[:], channels=128, reduce_op=bass_isa.ReduceOp.add
        )
        pref_ap = pref[:]
    nv = pool.tile([128, BPC], f32)
    nc.vector.tensor_scalar(
        out=nv[:], in0=pref_ap, scalar1=-1.0, scalar2=1.0,
        op0=mybir.AluOpType.add, op1=mybir.AluOpType.max,
    )
    # nm1 = n_valid - 1 = max(prefix - 2, 0), computed from pref directly so
    # it doesn't serialize behind nv
    nm1 = pool.tile([128, BPC], f32)
    nc.vector.tensor_scalar(
        out=nm1[:], in0=pref_ap, scalar1=-2.0, scalar2=0.0,
        op0=mybir.AluOpType.add, op1=mybir.AluOpType.max,
    )
    return nv, nm1


def _build_module_v2():
    from concourse import bacc, bass, bass_isa, mybir
    from concourse.tile import TileContext

    f32 = mybir.dt.float32
    i32 = mybir.dt.int32
    nc = bacc.Bacc("TRN2", target_bir_lowering=False, debug=False,
                   enable_asserts=False, num_devices=NCORES)
    tokens = nc.dram_tensor("tokens", [BPC, SEQ, HID], f32, kind="ExternalInput")
    mask = nc.dram_tensor("mask", [BPC, SEQ], i32, kind="ExternalInput")
    u_l = nc.dram_tensor("u_l", [128, FREE], f32, kind="ExternalInput")
    addc = nc.dram_tensor("addc", [128, FREE], f32, kind="ExternalInput")
    out = nc.dram_tensor("out", [BPC, HID], f32, kind="ExternalOutput")

    with TileContext(nc) as tc:
        with tc.tile_pool(name="p", bufs=1) as pool:
            nv, nm1 = _prefix_pipeline(nc, pool, None, mask, None, mybir, bass_isa)
            u_t = pool.tile([128, FREE], f32)
            nc.sync.dma_start(out=u_t[:], in_=u_l.ap())
            ac_t = pool.tile([128, FREE], f32)
            nc.sync.dma_start(out=ac_t[:], in_=addc.ap())

            def bcast(t):
                return t[:].rearrange("p (b o) -> p b o", o=1).to_broadcast(
                    [128, BPC, JW]
                )

            y = pool.tile([128, FREE], f32)
            nc.vector.tensor_tensor(
                out=y[:].rearrange("p (b t) -> p b t", b=BPC),
                in0=u_t[:].rearrange("p (b t) -> p b t", b=BPC),
                in1=bcast(nv), op=mybir.AluOpType.mult,
            )
            r = pool.tile([128, FREE], f32)
            nc.vector.tensor_scalar(
                out=r[:], in0=y[:], scalar1=TWO23, scalar2=-TWO23,
                op0=mybir.AluOpType.add, op1=mybir.AluOpType.add,
            )
            g = pool.tile([128, FREE], f32)
            nc.vector.tensor_tensor(out=g[:], in0=r[:], in1=y[:],
                                    op=mybir.AluOpType.is_gt)
            fl = pool.tile([128, FREE], f32)
            nc.vector.tensor_tensor(out=fl[:], in0=r[:], in1=g[:],
                                    op=mybir.AluOpType.subtract)
            im = pool.tile([128, FREE], f32)
            nc.vector.tensor_tensor(
                out=im[:].rearrange("p (b t) -> p b t", b=BPC),
                in0=fl[:].rearrange("p (b t) -> p b t", b=BPC),
                in1=bcast(nm1), op=mybir.AluOpType.min,
            )
            ef = pool.tile([128, FREE], f32)
            nc.vector.tensor_scalar_mul(out=ef[:], in0=im[:], scalar1=float(HID))
            ei_f = pool.tile([128, FREE], f32)
            nc.vector.tensor_tensor(out=ei_f[:], in0=ef[:], in1=ac_t[:],
                                    op=mybir.AluOpType.add)
            eidx = pool.tile([128, FREE], i32)
            nc.vector.tensor_copy(out=eidx[:], in_=ei_f[:])

            gt = pool.tile([128, FREE], f32)
            tok_flat = tokens.ap().rearrange("a b (c o) -> (a b c) o", o=1)
            for k in range(FREE):
                nc.gpsimd.indirect_dma_start(
                    out=gt[:, k : k + 1],
                    out_offset=None,
                    in_=tok_flat,
                    in_offset=bass.IndirectOffsetOnAxis(
                        ap=eidx[:, k : k + 1], axis=0
                    ),
                )
            nc.sync.dma_start(
                out=out.ap().rearrange("b (p t) -> p b t", t=JW),
                in_=gt[:].rearrange("p (b t) -> p b t", b=BPC),
            )
    nc.compile()
    return nc


def _v3_body(nc, tc, pool, psum_pool, tokens, mask, consts, out, mybir, bass_isa):
    f32 = mybir.dt.float32
    i16 = mybir.dt.int16
    COLS = NBLK * SBLK  # 256
    if True:  # (kept at historical indentation)
        if True:
            # one fused constant load: [u_r | badd | emask | ones]
            cst = pool.tile([128, 2 * COLS + CW + 128], f32)
            nc.sync.dma_start(out=cst[:], in_=consts.ap())
            u_t = cst[:, 0:COLS]
            ba_t = cst[:, COLS : 2 * COLS]
            m_t = cst[:, 2 * COLS : 2 * COLS + CW]
            ones128 = cst[:, 2 * COLS + CW : 2 * COLS + CW + 128]
            nv, nm1 = _prefix_pipeline(
                nc, pool, psum_pool, mask, ones128, mybir, bass_isa
            )

            # materialize n / n-1 in the 32-column period (col%32 = s, b=s//8):
            # n32[p, s] = nv[p, s//8]; ops then broadcast it across the 8 blocks
            n32 = pool.tile([128, SBLK], f32)
            nc.vector.tensor_copy(
                out=n32[:].rearrange("p (b t) -> p b t", b=BPC),
                in_=nv[:].rearrange("p (b o) -> p b o", o=1).to_broadcast(
                    [128, BPC, SBLK // BPC]
                ),
            )
            # nb32 = (n-1) + b*SEQ in the 32-column period (badd period is 32)
            nb32 = pool.tile([128, SBLK], f32)
            nc.vector.tensor_tensor(
                out=nb32[:].rearrange("p (b t) -> p b t", b=BPC),
                in0=nm1[:].rearrange("p (b o) -> p b o", o=1).to_broadcast(
                    [128, BPC, SBLK // BPC]
                ),
                in1=ba_t[:, :SBLK].rearrange("p (b t) -> p b t", b=BPC),
                op=mybir.AluOpType.add,
            )

            def brep(t):  # [128, 32] -> [128, NBLK, 32] (repeat across blocks)
                return t[:].rearrange("p (o s) -> p o s", o=1).to_broadcast(
                    [128, NBLK, SBLK]
                )

            def v3d(t):  # [128, 256] -> [128, NBLK, 32]
                return t[:].rearrange("p (blk s) -> p blk s", blk=NBLK)

            y = pool.tile([128, COLS], f32)
            nc.vector.tensor_tensor(out=v3d(y), in0=v3d(u_t), in1=brep(n32),
                                    op=mybir.AluOpType.mult)
            r = pool.tile([128, COLS], f32)
            nc.vector.tensor_scalar(
                out=r[:], in0=y[:], scalar1=TWO23, scalar2=-TWO23,
                op0=mybir.AluOpType.add, op1=mybir.AluOpType.add,
            )
            g = pool.tile([128, COLS], f32)
            nc.vector.tensor_tensor(out=g[:], in0=r[:], in1=y[:],
                                    op=mybir.AluOpType.is_gt)
            fl = pool.tile([128, COLS], f32)
            nc.vector.tensor_tensor(out=fl[:], in0=r[:], in1=g[:],
                                    op=mybir.AluOpType.subtract)
            # row = min(floor + b*SEQ, (n-1) + b*SEQ), emitted directly as
            # int16 (exact-integer convert) — the gather's index dtype
            fl2 = pool.tile([128, COLS], f32)
            nc.vector.tensor_tensor(out=fl2[:], in0=fl[:], in1=ba_t[:],
                                    op=mybir.AluOpType.add)
            rowf = pool.tile([128, COLS], f32)
            nc.vector.tensor_tensor(out=v3d(rowf), in0=v3d(fl2), in1=brep(nb32),
                                    op=mybir.AluOpType.min)
            t16 = pool.tile([128, COLS], i16)
            nc.vector.tensor_copy(out=t16[:], in_=rowf[:])

            # NBLK gathers, each followed by its own extraction so the DVE
            # work overlaps the Pool-serialized descriptor generation.
            # Extraction: multiply by the constant one-hot mask (cc == pd) and
            # reduce over the chunk dim.
            tok2d = tokens.ap().rearrange("a s h -> (a s) h")
            e_t = pool.tile([128, NBLK * NGRP], f32)
            mview = m_t[:].rearrange("p (o c) -> p o c", o=1).to_broadcast(
                [128, NGRP, CW]
            )
            for blk in range(NBLK):
                gk = pool.tile([128, NGRP * CW], f32, tag=f"g{blk}")
                nc.gpsimd.dma_gather(
                    out_ap=gk[:].rearrange("p (g c) -> p g c", c=CW),
                    in_ap=tok2d[:, blk * CW : (blk + 1) * CW],
                    idxs_ap=t16[:, blk * SBLK : (blk + 1) * SBLK],
                    num_idxs=NPAIR,
                    num_idxs_reg=NPAIR,
                    elem_size=CW,
                    elem_step=HID,
                )
                pk = pool.tile([128, NGRP * CW], f32, tag=f"pk{blk}")
                nc.vector.tensor_tensor(
                    out=pk[:].rearrange("p (g c) -> p g c", c=CW),
                    in0=gk[:].rearrange("p (g c) -> p g c", c=CW),
                    in1=mview,
                    op=mybir.AluOpType.mult,
                )
                nc.vector.tensor_reduce(
                    out=e_t[:, blk * NGRP : (blk + 1) * NGRP],
                    in_=pk[:].rearrange("p (g c) -> p g c", c=CW),
                    axis=mybir.AxisListType.X,
                    op=mybir.AluOpType.add,
                )
            # store: E[pd, blk*NGRP+b] -> out[b, blk*CW + pd]
            for b in range(BPC):
                nc.sync.dma_start(
                    out=out.ap()[b].rearrange("(blk pd) -> pd blk", pd=CW),
                    in_=e_t[:, b :: NGRP],
                )


def _build_module_v3(reps=1):
    from concourse import bacc, bass_isa, mybir
    from concourse.tile import TileContext

    f32 = mybir.dt.float32
    i32 = mybir.dt.int32
    nc = bacc.Bacc("TRN2", target_bir_lowering=False, debug=False,
                   enable_asserts=False, num_devices=NCORES)
    tokens = nc.dram_tensor("tokens", [BPC, SEQ, HID], f32, kind="ExternalInput")
    mask = nc.dram_tensor("mask", [BPC, SEQ], i32, kind="ExternalInput")
    # pair mapping within a block: pair i -> dest (pd = i%128, grp = i//128),
    # batch b = grp, column c = pd, j = blk*CW + pd.
    # index-tile slot (p0 = p%16, s = i//16):  i = s*16 + p0, so b = s//8 and
    # c = (s%8)*16 + p0 — b varies along a clean column axis (col = blk*32+s).
    # fused constants [u_r | badd | emask]:
    #   u_r[p, blk*SBLK+s]  = u[4*core + b(i), j(i, blk)]
    #   badd[p, blk*SBLK+s] = b(i) * SEQ
    #   emask[pd, cc]       = (cc == pd)
    consts = nc.dram_tensor(
        "consts", [128, 2 * NBLK * SBLK + CW + 128], f32, kind="ExternalInput"
    )
    out = nc.dram_tensor("out", [BPC, HID], f32, kind="ExternalOutput")

    with TileContext(nc) as tc:
        for _ in range(reps):
            with tc.tile_pool(name="p", bufs=1) as pool, tc.tile_pool(
                name="ps", bufs=1, space="PSUM"
            ) as psum_pool:
                _v3_body(
                    nc, tc, pool, psum_pool, tokens, mask, consts, out,
                    mybir, bass_isa,
                )
    nc.compile()
    return nc


def _get_module():
    if "nc" not in _CACHE:
        _CACHE["nc"] = _build_module_v3() if KVER == 3 else _build_module_v2()
    return _CACHE["nc"]


def _u_const():
    # Input-independent sampling constant; bit-identical to the reference's
    # jax.random.uniform(key(42), ...) on any backend (threefry is
    # platform-deterministic).
    if "u" not in _CACHE:
        import jax

        with jax.default_device(jax.devices("cpu")[0]):
            u = jax.random.uniform(
                jax.random.key(42), (BS, HID), dtype="float32"
            )
            _CACHE["u"] = np.asarray(u)
    return _CACHE["u"]


def _consts_v2():
    if "addc" not in _CACHE:
        b = np.arange(BPC, dtype=np.int64)[None, :, None] * (SEQ * HID)
        j = (
            np.arange(128, dtype=np.int64)[:, None, None] * JW
            + np.arange(JW, dtype=np.int64)[None, None, :]
        )
        _CACHE["addc"] = (b + j).reshape(128, FREE).astype(np.float32)
    return _CACHE["addc"]


def _pair_decode(p, col):
    """Index-tile slot (partition p, column col=blk*SBLK+s) -> (b, j)."""
    blk, s = col // SBLK, col % SBLK
    i = s * 16 + (p % 16)
    pd, grp = i % 128, i // 128
    return grp, blk * CW + pd


def _consts_v3():
    if "badd" not in _CACHE:
        p = np.arange(128)[:, None]
        col = np.arange(NBLK * SBLK)[None, :]
        b, _ = _pair_decode(p, col)
        _CACHE["badd"] = (b * SEQ).astype(np.float32)
        pd = np.arange(128)[:, None]
        cc = np.arange(CW)[None, :]
        _CACHE["emask"] = (cc == pd).astype(np.float32)
    return _CACHE["badd"], _CACHE["emask"]


def _u_r_for_core(u, core):
    p = np.arange(128)[:, None]
    col = np.arange(NBLK * SBLK)[None, :]
    b, j = _pair_decode(p, col)
    return np.ascontiguousarray(u[4 * core + b, j].astype(np.float32))


def build_in_maps(tokens, mask):
    u = _u_const()
    in_maps = []
    for c in range(NCORES):
        sl = slice(c * BPC, (c + 1) * BPC)
        m = {
            "tokens": np.ascontiguousarray(tokens[sl], dtype=np.float32),
            "mask": np.ascontiguousarray(mask[sl], dtype=np.int32),
        }
        if KVER == 3:
            badd, emask = _consts_v3()
            ones = np.ones((128, 128), np.float32)
            m["consts"] = np.ascontiguousarray(
                np.concatenate([_u_r_for_core(u, c), badd, emask, ones], axis=1)
            )
        else:
            u_c = (
                u[sl].reshape(BPC, 128, JW).transpose(1, 0, 2).reshape(128, FREE)
            )
            m["u_l"] = np.ascontiguousarray(u_c)
            m["addc"] = _consts_v2()
        in_maps.append(m)
    return in_maps


def _get_runner():
    """Build the 8-core shard_map jit once; returns f(in_maps) -> [out_c]."""
    if "runner" in _CACHE:
        return _CACHE["runner"]
    import jax
    from jax.sharding import Mesh, NamedSharding, PartitionSpec
    from jax.experimental.shard_map import shard_map
    from concourse import mybir
    from concourse.bass2jax import (
        _bass_exec_p,
        install_neuronx_cc_hook,
        partition_id_tensor,
    )

    nc = _get_module()
    install_neuronx_cc_hook()
    partition_name = nc.partition_id_tensor.name if nc.partition_id_tensor else None
    in_names, out_names, out_avals, zero_outs = [], [], [], []
    for alloc in nc.m.functions[0].allocations:
        if type(alloc).__name__ != "MemoryLocationSet":
            continue
        name = alloc.memorylocations[0].name
        if alloc.kind == "ExternalInput":
            if name != partition_name:
                in_names.append(name)
        elif alloc.kind == "ExternalOutput":
            out_names.append(name)
            shape = tuple(alloc.tensor_shape)
            dtype = mybir.dt.np(alloc.dtype)
            out_avals.append(jax.core.ShapedArray(shape, dtype))
            zero_outs.append(np.zeros(shape, dtype))
    n_params = len(in_names)
    all_in_names = in_names + out_names
    if partition_name is not None:
        all_in_names = all_in_names + [partition_name]

    def _body(*args):
        operands = list(args)
        if partition_name is not None:
            operands.append(partition_id_tensor())
        return tuple(
            _bass_exec_p.bind(
                *operands,
                out_avals=tuple(out_avals),
                in_names=tuple(all_in_names),
                out_names=tuple(out_names),
                lowering_input_output_aliases=(),
                sim_require_finite=True,
                sim_require_nnan=True,
                nc=nc,
            )
        )

    devices = jax.devices()[:NCORES]
    mesh = Mesh(np.asarray(devices), ("core",))
    n_outs = len(out_names)
    sharded = jax.jit(
        shard_map(
            _body,
            mesh=mesh,
            in_specs=(PartitionSpec("core"),) * (n_params + n_outs),
            out_specs=(PartitionSpec("core"),) * n_outs,
            check_rep=False,
        ),
        donate_argnums=tuple(range(n_params, n_params + n_outs)),
        keep_unused=True,
    )
    sh = NamedSharding(mesh, PartitionSpec("core"))

    def run(in_maps):
        concat_in = [
            np.concatenate(
                [np.asarray(in_maps[c][nm]) for c in range(NCORES)], axis=0
            )
            for nm in in_names
        ]
        zeros = [
            np.zeros((NCORES * z.shape[0], *z.shape[1:]), z.dtype)
            for z in zero_outs
        ]
        out_arrs = sharded(*concat_in, *zeros)
        oi = out_names.index("out")
        full = np.asarray(out_arrs[oi])
        return full.reshape(NCORES, *out_avals[oi].shape)

    _CACHE["runner_parts"] = {
        "sharded": sharded,
        "in_names": in_names,
        "out_names": out_names,
        "zero_outs": zero_outs,
        "mesh": mesh,
    }
    _CACHE["runner"] = run
    return run


def kernel(output_tokens_from_bert, attention_mask):
    run = _get_runner()
    in_maps = build_in_maps(output_tokens_from_bert, attention_mask)
    per_core = run(in_maps)
    return np.concatenate(list(per_core), axis=0).astype(np.float32)


# revision 40
# speedup vs baseline: 1.2664x; 1.1259x over previous
"""Trainium2 Bass kernel for nn_CondensedEmbracementLayer.

Computation (per batch element b):
  prefix[b]  = number of leading 1s in attention_mask[b]  (contiguous-prefix mask)
  n_valid[b] = max(prefix[b] - 1, 1)
  u          = jax.random.uniform(key(42), (32, 1024))      # input-independent constant
  idx[b, j]  = min(trunc_f32(u[b, j] * n_valid[b]), n_valid[b] - 1)
  out[b, j]  = tokens[b, idx[b, j], j]

Strategy: pure data parallel over batch — 4 batch elements per NeuronCore x 8
cores. On each core the mask is popcounted on-chip (free-dim reduce +
partition all-reduce), the sample row indices are computed in f32 exactly
(all intermediates are integers < 2^24; floor via the +2^23 round trick plus
an is_gt fixup so it bit-matches jax's truncation), and the 4096 needed
elements are fetched directly from HBM with indirect/gather DMAs — the 64MB
token shard is never streamed.

Two device-side gather flavours (BASS_KERNEL_V, default 3):
  v2: 32 x indirect_dma_start, one [128,1] element-gather per column
      (the HW ucode emits exactly one descriptor per partition).
  v3: 8 x dma_gather, 512 row-chunks of 128 f32 each per instruction
      (int16 row indices wrapped in 16 partitions), each followed by a
      constant-one-hot multiply + free-dim reduce that extracts the wanted
      element per chunk while the next gather's descriptors generate.
      The SWDGE scratch ring is enlarged to 128KB so consecutive gathers'
      descriptors pipeline instead of serializing on ring space, and the
      cross-partition popcount reduce+broadcast is a single ones-matmul on
      the otherwise idle TensorE.
"""

import os
import numpy as np

BS, SEQ, HID = 32, 4096, 1024
NCORES = 8
BPC = BS // NCORES          # batch elements per core
JW = HID // 128             # j-values per partition per batch element (v2)
FREE = BPC * JW             # free-dim columns of the v2 work tile
TWO23 = float(2 << 22)

# v3 geometry: 8 column blocks of 128; per block 512 (b, j) pairs.
# NGRP == BPC lets the pair mapping be (b = grp, c = pd), which keeps every
# access pattern at <=3 dims (walrus codegen can't encode 4-D+ patterns).
NBLK = 8
CW = HID // NBLK            # 128 columns per block
NPAIR = BPC * CW            # 512 gathered rows per block
NGRP = NPAIR // 128         # 4 dest partition groups per block (== BPC)
SBLK = NPAIR // 16          # 32 index-tile columns per block

KVER = int(os.environ.get("BASS_KERNEL_V", "3"))
# gather drain-pipelining experiment: standard-size DMA packets and two
# SWDGE queues so one gather's descriptor drain overlaps the next's gen
MULTIQ = os.environ.get("BASS_KERNEL_MULTIQ", "1") == "1"

_CACHE = {}


def _prefix_pipeline(nc, pool, psum_pool, mask, ones128, mybir, bass_isa):
    """mask [BPC, SEQ] -> nv, nm1 tiles [128, BPC] (broadcast to all parts).

    The cross-partition popcount reduction + broadcast is one ones-matmul on
    the otherwise idle TensorE: out[p, b] = sum_k ones[k, p] * part[k, b]
    (gpsimd partition_all_reduce measured ~10x slower on HW).
    """
    f32 = mybir.dt.float32
    i32 = mybir.dt.int32
    # partition p holds seq chunk [p*32, (p+1)*32) of each batch row, so the
    # load uses contiguous 128B runs instead of 4B-strided descriptors
    m_i = pool.tile([128, BPC * 32], i32)
    nc.sync.dma_start(
        out=m_i[:].rearrange("p (b c) -> p b c", b=BPC),
        in_=mask.ap().rearrange("b (p c) -> p b c", p=128),
    )
    m_f = pool.tile([128, BPC * 32], f32)
    nc.vector.tensor_copy(out=m_f[:], in_=m_i[:])
    part = pool.tile([128, BPC], f32)
    nc.vector.tensor_reduce(
        out=part[:],
        in_=m_f[:].rearrange("p (b c) -> p b c", b=BPC),
        axis=mybir.AxisListType.X,
        op=mybir.AluOpType.add,
    )
    if ones128 is not None:
        pref_ps = psum_pool.tile([128, BPC], f32, space="PSUM")
        nc.tensor.matmul(
            out=pref_ps[:], lhsT=ones128, rhs=part[:], start=True, stop=True
        )
        pref_ap = pref_ps[:]
    else:
        pref = pool.tile([128, BPC], f32)
        nc.gpsimd.partition_all_reduce(
            pref[:], part[:], channels=128, reduce_op=bass_isa.ReduceOp.add
        )
        pref_ap = pref[:]
    nv = pool.tile([128, BPC], f32)
    nc.vector.tensor_scalar(
        out=nv[:], in0=pref_ap, scalar1=-1.0, scalar2=1.0,
        op0=mybir.AluOpType.add, op1=mybir.AluOpType.max,
    )
    # nm1 = n_valid - 1 = max(prefix - 2, 0), computed from pref directly so
    # it doesn't serialize behind nv
    nm1 = pool.tile([128, BPC], f32)
    nc.vector.tensor_scalar(
        out=nm1[:], in0=pref_ap, scalar1=-2.0, scalar2=0.0,
        op0=mybir.AluOpType.add, op1=mybir.AluOpType.max,
    )
    return nv, nm1


def _build_module_v2():
    from concourse import bacc, bass, bass_isa, mybir
    from concourse.tile import TileContext

    f32 = mybir.dt.float32
    i32 = mybir.dt.int32
    nc = bacc.Bacc("TRN2", target_bir_lowering=False, debug=False,
                   enable_asserts=False, num_devices=NCORES)
    tokens = nc.dram_tensor("tokens", [BPC, SEQ, HID], f32, kind="ExternalInput")
    mask = nc.dram_tensor("mask", [BPC, SEQ], i32, kind="ExternalInput")
    u_l = nc.dram_tensor("u_l", [128, FREE], f32, kind="ExternalInput")
    addc = nc.dram_tensor("addc", [128, FREE], f32, kind="ExternalInput")
    out = nc.dram_tensor("out", [BPC, HID], f32, kind="ExternalOutput")

    with TileContext(nc) as tc:
        with tc.tile_pool(name="p", bufs=1) as pool:
            nv, nm1 = _prefix_pipeline(nc, pool, None, mask, None, mybir, bass_isa)
            u_t = pool.tile([128, FREE], f32)
            nc.sync.dma_start(out=u_t[:], in_=u_l.ap())
            ac_t = pool.tile([128, FREE], f32)
            nc.sync.dma_start(out=ac_t[:], in_=addc.ap())

            def bcast(t):
                return t[:].rearrange("p (b o) -> p b o", o=1).to_broadcast(
                    [128, BPC, JW]
                )

            y = pool.tile([128, FREE], f32)
            nc.vector.tensor_tensor(
                out=y[:].rearrange("p (b t) -> p b t", b=BPC),
                in0=u_t[:].rearrange("p (b t) -> p b t", b=BPC),
                in1=bcast(nv), op=mybir.AluOpType.mult,
            )
            r = pool.tile([128, FREE], f32)
            nc.vector.tensor_scalar(
                out=r[:], in0=y[:], scalar1=TWO23, scalar2=-TWO23,
                op0=mybir.AluOpType.add, op1=mybir.AluOpType.add,
            )
            g = pool.tile([128, FREE], f32)
            nc.vector.tensor_tensor(out=g[:], in0=r[:], in1=y[:],
                                    op=mybir.AluOpType.is_gt)
            fl = pool.tile([128, FREE], f32)
            nc.vector.tensor_tensor(out=fl[:], in0=r[:], in1=g[:],
                                    op=mybir.AluOpType.subtract)
            im = pool.tile([128, FREE], f32)
            nc.vector.tensor_tensor(
                out=im[:].rearrange("p (b t) -> p b t", b=BPC),
                in0=fl[:].rearrange("p (b t) -> p b t", b=BPC),
                in1=bcast(nm1), op=mybir.AluOpType.min,
            )
            ef = pool.tile([128, FREE], f32)
            nc.vector.tensor_scalar_mul(out=ef[:], in0=im[:], scalar1=float(HID))
            ei_f = pool.tile([128, FREE], f32)
            nc.vector.tensor_tensor(out=ei_f[:], in0=ef[:], in1=ac_t[:],
                                    op=mybir.AluOpType.add)
            eidx = pool.tile([128, FREE], i32)
            nc.vector.tensor_copy(out=eidx[:], in_=ei_f[:])

            gt = pool.tile([128, FREE], f32)
            tok_flat = tokens.ap().rearrange("a b (c o) -> (a b c) o", o=1)
            for k in range(FREE):
                nc.gpsimd.indirect_dma_start(
                    out=gt[:, k : k + 1],
                    out_offset=None,
                    in_=tok_flat,
                    in_offset=bass.IndirectOffsetOnAxis(
                        ap=eidx[:, k : k + 1], axis=0
                    ),
                )
            nc.sync.dma_start(
                out=out.ap().rearrange("b (p t) -> p b t", t=JW),
                in_=gt[:].rearrange("p (b t) -> p b t", b=BPC),
            )
    nc.compile()
    return nc


def _v3_body(nc, tc, pool, psum_pool, tokens, mask, consts, out, mybir, bass_isa):
    f32 = mybir.dt.float32
    i16 = mybir.dt.int16
    COLS = NBLK * SBLK  # 256
    if True:  # (kept at historical indentation)
        if True:
            # one fused constant load: [u_r | badd | emask | ones]
            cst = pool.tile([128, 2 * COLS + CW + 128], f32)
            nc.sync.dma_start(out=cst[:], in_=consts.ap())
            u_t = cst[:, 0:COLS]
            ba_t = cst[:, COLS : 2 * COLS]
            m_t = cst[:, 2 * COLS : 2 * COLS + CW]
            ones128 = cst[:, 2 * COLS + CW : 2 * COLS + CW + 128]
            nv, nm1 = _prefix_pipeline(
                nc, pool, psum_pool, mask, ones128, mybir, bass_isa
            )

            # materialize n / n-1 in the 32-column period (col%32 = s, b=s//8):
            # n32[p, s] = nv[p, s//8]; ops then broadcast it across the 8 blocks
            n32 = pool.tile([128, SBLK], f32)
            nc.vector.tensor_copy(
                out=n32[:].rearrange("p (b t) -> p b t", b=BPC),
                in_=nv[:].rearrange("p (b o) -> p b o", o=1).to_broadcast(
                    [128, BPC, SBLK // BPC]
                ),
            )
            # nb32 = (n-1) + b*SEQ in the 32-column period (badd period is 32)
            nb32 = pool.tile([128, SBLK], f32)
            nc.vector.tensor_tensor(
                out=nb32[:].rearrange("p (b t) -> p b t", b=BPC),
                in0=nm1[:].rearrange("p (b o) -> p b o", o=1).to_broadcast(
                    [128, BPC, SBLK // BPC]
                ),
                in1=ba_t[:, :SBLK].rearrange("p (b t) -> p b t", b=BPC),
                op=mybir.AluOpType.add,
            )

            def brep(t):  # [128, 32] -> [128, NBLK, 32] (repeat across blocks)
                return t[:].rearrange("p (o s) -> p o s", o=1).to_broadcast(
                    [128, NBLK, SBLK]
                )

            def v3d(t):  # [128, 256] -> [128, NBLK, 32]
                return t[:].rearrange("p (blk s) -> p blk s", blk=NBLK)

            y = pool.tile([128, COLS], f32)
            nc.vector.tensor_tensor(out=v3d(y), in0=v3d(u_t), in1=brep(n32),
                                    op=mybir.AluOpType.mult)
            r = pool.tile([128, COLS], f32)
            nc.vector.tensor_scalar(
                out=r[:], in0=y[:], scalar1=TWO23, scalar2=-TWO23,
                op0=mybir.AluOpType.add, op1=mybir.AluOpType.add,
            )
            g = pool.tile([128, COLS], f32)
            nc.vector.tensor_tensor(out=g[:], in0=r[:], in1=y[:],
                                    op=mybir.AluOpType.is_gt)
            fl = pool.tile([128, COLS], f32)
            nc.vector.tensor_tensor(out=fl[:], in0=r[:], in1=g[:],
                                    op=mybir.AluOpType.subtract)
            # row = min(floor + b*SEQ, (n-1) + b*SEQ), emitted directly as
            # int16 (exact-integer convert) — the gather's index dtype
            fl2 = pool.tile([128, COLS], f32)
            nc.vector.tensor_tensor(out=fl2[:], in0=fl[:], in1=ba_t[:],
                                    op=mybir.AluOpType.add)
            rowf = pool.tile([128, COLS], f32)
            nc.vector.tensor_tensor(out=v3d(rowf), in0=v3d(fl2), in1=brep(nb32),
                                    op=mybir.AluOpType.min)
            t16 = pool.tile([128, COLS], i16)
            nc.vector.tensor_copy(out=t16[:], in_=rowf[:])

            # NBLK gathers, each followed by its own extraction so the DVE
            # work overlaps the Pool-serialized descriptor generation.
            # Extraction: multiply by the constant one-hot mask (cc == pd) and
            # reduce over the chunk dim.
            tok2d = tokens.ap().rearrange("a s h -> (a s) h")
            e_t = pool.tile([128, NBLK * NGRP], f32)
            mview = m_t[:].rearrange("p (o c) -> p o c", o=1).to_broadcast(
                [128, NGRP, CW]
            )
            for blk in range(NBLK):
                gk = pool.tile([128, NGRP * CW], f32, tag=f"g{blk}")
                nc.gpsimd.dma_gather(
                    out_ap=gk[:].rearrange("p (g c) -> p g c", c=CW),
                    in_ap=tok2d[:, blk * CW : (blk + 1) * CW],
                    idxs_ap=t16[:, blk * SBLK : (blk + 1) * SBLK],
                    num_idxs=NPAIR,
                    num_idxs_reg=NPAIR,
                    elem_size=CW,
                    elem_step=HID,
                    single_packet=not MULTIQ,
                    queue_num=(blk % 2) if MULTIQ else 0,
                )
                pk = pool.tile([128, NGRP * CW], f32, tag=f"pk{blk}")
                nc.vector.tensor_tensor(
                    out=pk[:].rearrange("p (g c) -> p g c", c=CW),
                    in0=gk[:].rearrange("p (g c) -> p g c", c=CW),
                    in1=mview,
                    op=mybir.AluOpType.mult,
                )
                nc.vector.tensor_reduce(
                    out=e_t[:, blk * NGRP : (blk + 1) * NGRP],
                    in_=pk[:].rearrange("p (g c) -> p g c", c=CW),
                    axis=mybir.AxisListType.X,
                    op=mybir.AluOpType.add,
                )
            # store: E[pd, blk*NGRP+b] -> out[b, blk*CW + pd]
            for b in range(BPC):
                nc.sync.dma_start(
                    out=out.ap()[b].rearrange("(blk pd) -> pd blk", pd=CW),
                    in_=e_t[:, b :: NGRP],
                )


def _build_module_v3(reps=1):
    from concourse import bacc, bass_isa, mybir
    from concourse.tile import TileContext

    f32 = mybir.dt.float32
    i32 = mybir.dt.int32
    # 128KB SWDGE scratch: the default 16KB ring holds exactly one gather's
    # 1024 descriptors, serializing each gather's generation behind the
    # previous one's drain; 8x lets all 8 gathers pipeline. Two SWDGE queues
    # (gathers alternate) decouple the two FIFOs further.
    nc = bacc.Bacc("TRN2", target_bir_lowering=False, debug=False,
                   enable_asserts=False, num_devices=NCORES,
                   dynamic_dma_scratch_size=1 << 17,
                   num_swdge_queues=2 if MULTIQ else 1)
    tokens = nc.dram_tensor("tokens", [BPC, SEQ, HID], f32, kind="ExternalInput")
    mask = nc.dram_tensor("mask", [BPC, SEQ], i32, kind="ExternalInput")
    # pair mapping within a block: pair i -> dest (pd = i%128, grp = i//128),
    # batch b = grp, column c = pd, j = blk*CW + pd.
    # index-tile slot (p0 = p%16, s = i//16):  i = s*16 + p0, so b = s//8 and
    # c = (s%8)*16 + p0 — b varies along a clean column axis (col = blk*32+s).
    # fused constants [u_r | badd | emask]:
    #   u_r[p, blk*SBLK+s]  = u[4*core + b(i), j(i, blk)]
    #   badd[p, blk*SBLK+s] = b(i) * SEQ
    #   emask[pd, cc]       = (cc == pd)
    consts = nc.dram_tensor(
        "consts", [128, 2 * NBLK * SBLK + CW + 128], f32, kind="ExternalInput"
    )
    out = nc.dram_tensor("out", [BPC, HID], f32, kind="ExternalOutput")

    with TileContext(nc) as tc:
        for _ in range(reps):
            with tc.tile_pool(name="p", bufs=1) as pool, tc.tile_pool(
                name="ps", bufs=1, space="PSUM"
            ) as psum_pool:
                _v3_body(
                    nc, tc, pool, psum_pool, tokens, mask, consts, out,
                    mybir, bass_isa,
                )
    nc.compile()
    return nc


def _get_module():
    if "nc" not in _CACHE:
        if KVER == 3:
            try:
                _CACHE["nc"] = _build_module_v3()
                _CACHE["ver"] = 3
            except Exception:
                _CACHE["nc"] = _build_module_v2()
                _CACHE["ver"] = 2
        else:
            _CACHE["nc"] = _build_module_v2()
            _CACHE["ver"] = 2
    return _CACHE["nc"]


def _u_const():
    # Input-independent sampling constant; bit-identical to the reference's
    # jax.random.uniform(key(42), ...) on any backend (threefry is
    # platform-deterministic).
    if "u" not in _CACHE:
        import jax

        with jax.default_device(jax.devices("cpu")[0]):
            u = jax.random.uniform(
                jax.random.key(42), (BS, HID), dtype="float32"
            )
            _CACHE["u"] = np.asarray(u)
    return _CACHE["u"]


def _consts_v2():
    if "addc" not in _CACHE:
        b = np.arange(BPC, dtype=np.int64)[None, :, None] * (SEQ * HID)
        j = (
            np.arange(128, dtype=np.int64)[:, None, None] * JW
            + np.arange(JW, dtype=np.int64)[None, None, :]
        )
        _CACHE["addc"] = (b + j).reshape(128, FREE).astype(np.float32)
    return _CACHE["addc"]


def _pair_decode(p, col):
    """Index-tile slot (partition p, column col=blk*SBLK+s) -> (b, j)."""
    blk, s = col // SBLK, col % SBLK
    i = s * 16 + (p % 16)
    pd, grp = i % 128, i // 128
    return grp, blk * CW + pd


def _consts_v3():
    if "badd" not in _CACHE:
        p = np.arange(128)[:, None]
        col = np.arange(NBLK * SBLK)[None, :]
        b, _ = _pair_decode(p, col)
        _CACHE["badd"] = (b * SEQ).astype(np.float32)
        pd = np.arange(128)[:, None]
        cc = np.arange(CW)[None, :]
        _CACHE["emask"] = (cc == pd).astype(np.float32)
    return _CACHE["badd"], _CACHE["emask"]


def _u_r_for_core(u, core):
    p = np.arange(128)[:, None]
    col = np.arange(NBLK * SBLK)[None, :]
    b, j = _pair_decode(p, col)
    return np.ascontiguousarray(u[4 * core + b, j].astype(np.float32))


def build_in_maps(tokens, mask):
    u = _u_const()
    ver = _CACHE.get("ver", KVER)
    in_maps = []
    for c in range(NCORES):
        sl = slice(c * BPC, (c + 1) * BPC)
        m = {
            "tokens": np.ascontiguousarray(tokens[sl], dtype=np.float32),
            "mask": np.ascontiguousarray(mask[sl], dtype=np.int32),
        }
        if ver == 3:
            badd, emask = _consts_v3()
            ones = np.ones((128, 128), np.float32)
            m["consts"] = np.ascontiguousarray(
                np.concatenate([_u_r_for_core(u, c), badd, emask, ones], axis=1)
            )
        else:
            u_c = (
                u[sl].reshape(BPC, 128, JW).transpose(1, 0, 2).reshape(128, FREE)
            )
            m["u_l"] = np.ascontiguousarray(u_c)
            m["addc"] = _consts_v2()
        in_maps.append(m)
    return in_maps


def _get_runner():
    """Build the 8-core shard_map jit once; returns f(in_maps) -> [out_c]."""
    if "runner" in _CACHE:
        return _CACHE["runner"]
    import jax
    from jax.sharding import Mesh, NamedSharding, PartitionSpec
    from jax.experimental.shard_map import shard_map
    from concourse import mybir
    from concourse.bass2jax import (
        _bass_exec_p,
        install_neuronx_cc_hook,
        partition_id_tensor,
    )

    nc = _get_module()
    install_neuronx_cc_hook()
    partition_name = nc.partition_id_tensor.name if nc.partition_id_tensor else None
    in_names, out_names, out_avals, zero_outs = [], [], [], []
    for alloc in nc.m.functions[0].allocations:
        if type(alloc).__name__ != "MemoryLocationSet":
            continue
        name = alloc.memorylocations[0].name
        if alloc.kind == "ExternalInput":
            if name != partition_name:
                in_names.append(name)
        elif alloc.kind == "ExternalOutput":
            out_names.append(name)
            shape = tuple(alloc.tensor_shape)
            dtype = mybir.dt.np(alloc.dtype)
            out_avals.append(jax.core.ShapedArray(shape, dtype))
            zero_outs.append(np.zeros(shape, dtype))
    n_params = len(in_names)
    all_in_names = in_names + out_names
    if partition_name is not None:
        all_in_names = all_in_names + [partition_name]

    def _body(*args):
        operands = list(args)
        if partition_name is not None:
            operands.append(partition_id_tensor())
        return tuple(
            _bass_exec_p.bind(
                *operands,
                out_avals=tuple(out_avals),
                in_names=tuple(all_in_names),
                out_names=tuple(out_names),
                lowering_input_output_aliases=(),
                sim_require_finite=True,
                sim_require_nnan=True,
                nc=nc,
            )
        )

    devices = jax.devices()[:NCORES]
    mesh = Mesh(np.asarray(devices), ("core",))
    n_outs = len(out_names)
    sharded = jax.jit(
        shard_map(
            _body,
            mesh=mesh,
            in_specs=(PartitionSpec("core"),) * (n_params + n_outs),
            out_specs=(PartitionSpec("core"),) * n_outs,
            check_rep=False,
        ),
        donate_argnums=tuple(range(n_params, n_params + n_outs)),
        keep_unused=True,
    )
    sh = NamedSharding(mesh, PartitionSpec("core"))

    def run(in_maps):
        concat_in = [
            np.concatenate(
                [np.asarray(in_maps[c][nm]) for c in range(NCORES)], axis=0
            )
            for nm in in_names
        ]
        zeros = [
            np.zeros((NCORES * z.shape[0], *z.shape[1:]), z.dtype)
            for z in zero_outs
        ]
        out_arrs = sharded(*concat_in, *zeros)
        oi = out_names.index("out")
        full = np.asarray(out_arrs[oi])
        return full.reshape(NCORES, *out_avals[oi].shape)

    _CACHE["runner_parts"] = {
        "sharded": sharded,
        "in_names": in_names,
        "out_names": out_names,
        "zero_outs": zero_outs,
        "mesh": mesh,
    }
    _CACHE["runner"] = run
    return run


def kernel(output_tokens_from_bert, attention_mask):
    run = _get_runner()
    in_maps = build_in_maps(output_tokens_from_bert, attention_mask)
    per_core = run(in_maps)
    return np.concatenate(list(per_core), axis=0).astype(np.float32)
